# revision 8
# baseline (speedup 1.0000x reference)
"""Binary conv1d + maxpool + per-channel threshold, Trainium2 Bass kernel.

Problem (hardcoded shapes):
  I:  [64, 64, 16384] f32   -> pad L by (3,3) with -1.0, sign()
  W:  [128, 64, 7]    f32   -> sign()
  conv1d (VALID over padded) -> [64, 128, 16384]
  maxpool1d(k=7, s=2)        -> [64, 128, 8189]
  per-channel threshold      -> +-sign outputs

Sharding: data-parallel over batch, 8 batches per core on 8 cores.

v2 device algorithm (unit-threshold fast path), per core (4 batch pairs):
  - Binarize to fp8 parity tiles SE/SO (batch pair stacked on 128
    partitions).  Two pairs binarize on ScalarE (Sign, +-1); two pairs on
    DVE (is_ge, {0,1}) to balance engine load.  The {0,1} encoding is
    corrected via a per-channel threshold shift (conv = 2*conv01 - W1).
  - Conv: fp8 DoubleRow matmuls, 2 taps per MM via pair-stride-2 rhs APs
    (taps {0,4},{2,6},{1,5},{3,zero}); 4 weight pairs x 2 parities x 2
    512-col windows per 2048-col super-region, 2 batches concurrent on
    the PE row-tile halves.  PSUM is one manual [128,4096] f32 tile,
    halves alternate so evacuation overlaps the other half's matmuls.
  - Evac fuses the threshold: ScalarE Sign(psum + bias) -> +-1 bf16 into
    CE/CO tiles (threshold commutes with max since f is monotone for the
    unit params).  2048-wide evac ops with a parity-splitting 4D out AP.
  - Pool: T[i] = max(ce[i], co[i], ce[i+1]) as 2 DVE 2048-col TT maxes;
    tail out[l] = max(T[l],T[l+1],T[l+2]) as 2 full-width TT maxes.
    All +-1 bf16, so the pooled value IS the final output (no threshold
    pass, no final map).
  - Output bf16, cast to f32 on host.
"""

import numpy as np

B, Cin, L = 64, 64, 16384
Cout, K = 128, 7
PAD = 3
LPAD = L + 2 * PAD          # 16390
Lp = (L - 7) // 2 + 1       # 8189
NCORES = 8
BPC = B // NCORES           # 8 batches per core
PAIRS = BPC // 2            # 4

SW = 8196                   # SE/SO fp8 tile width (8195 data + 1 pad)
CW = 8200                   # CE/CO region pitch in the CEO tile
NT = 8192                   # T-buffer slots per batch
NREG = 8                    # 2048-conv-col super-regions per batch
SIGN_CHUNK = 4096
ACT_BIN_PAIRS = 2           # pairs 0..1 binarize on ScalarE (+-1)

_CACHE = {}


def _build_v2():
    import concourse.mybir as mybir
    import bass_rust
    from concourse import bacc
    from concourse.bass import AP
    from concourse.tile import TileContext

    f32 = mybir.dt.float32
    bf16 = mybir.dt.bfloat16
    fp8 = mybir.dt.float8e4
    AF = mybir.ActivationFunctionType
    OP = mybir.AluOpType
    PM = mybir.MatmulPerfMode

    nc = bacc.Bacc()
    I_in = nc.declare_dram_parameter("I", [BPC, Cin, L], f32, isOutput=False)
    # W host layout: [Cin, 7 slots x 128] f32, slot order t0,t4,t2,t6,t1,t5,t3
    W_in = nc.declare_dram_parameter("W", [Cin, 896], f32, isOutput=False)
    # thr: col0 = -tp, col1 = -0.5*tp
    thr_in = nc.declare_dram_parameter("thr", [Cout, 8], f32, isOutput=False)
    O_out = nc.declare_dram_parameter("O", [BPC, Cout, Lp], bf16, isOutput=True)

    with TileContext(nc) as tc:
        with (
            tc.tile_pool(name="wpool", bufs=1) as wpool,
            tc.tile_pool(name="fpool", bufs=2) as fpool,
            tc.tile_pool(name="spool", bufs=2) as spool,
            tc.tile_pool(name="cpool", bufs=2) as cpool,
            tc.tile_pool(name="xpool", bufs=1) as xpool,
            tc.tile_pool(name="tpool", bufs=2) as tpool,
            tc.tile_pool(name="vpool", bufs=1) as vpool,
            tc.tile_pool(name="opool", bufs=1) as opool,
            tc.tile_pool(name="pspool", bufs=1, space="PSUM") as pspool,
        ):
            # ---- weights: sign -> fp8, 7 real slots + 1 zero slot
            wf = fpool.tile([128, 896], f32, tag="WF")
            nc.sync.dma_start(out=wf[0:64, 0:896], in_=W_in[:])
            nc.sync.dma_start(out=wf[64:128, 0:896], in_=W_in[:])
            w8 = wpool.tile([128, 1024], fp8, tag="w8")
            nc.scalar.activation(out=w8[:, 0:896], in_=wf[:, 0:896],
                                 func=AF.Sign)
            nc.vector.memset(w8[:, 896:1024], 0.0)

            thr = wpool.tile([128, 8], f32, tag="thr")
            nc.sync.dma_start(out=thr[:, :], in_=thr_in[:])

            ones8 = wpool.tile([128, 1], fp8, tag="ones8")
            nc.vector.memset(ones8[:, :], 1.0)

            PS = pspool.tile([128, 4096], f32, tag="PS", name="PS")

            # ---- W1[co] = sum_{ci,k} sign(w): 7 accumulating 1-col matmuls
            for k in range(7):
                nc.tensor.matmul(PS[:, 0:1],
                                 w8[0:64, 128 * k:128 * (k + 1)],
                                 ones8[0:64, 0:1],
                                 start=(k == 0), stop=(k == 6))
            # bias_d = -0.5*W1 - 0.5*tp  (evac bias for {0,1}-encoded pairs)
            bias_d = wpool.tile([128, 1], f32, tag="bias_d")
            nc.vector.tensor_scalar(out=bias_d[:, :], in0=PS[:, 0:1],
                                    scalar1=-0.5, scalar2=thr[:, 1:2],
                                    op0=OP.mult, op1=OP.add)

            # weight slot layout: pair P cols [256P : 256P+256] as [2 x 128]
            # P0=(t0,t4) slots 0,1; P1=(t2,t6) slots 2,3; P2=(t1,t5) slots
            # 4,5; P3=(t3,zero) slots 6,7.

            def rhs_ap(S, h, base):
                # sliding tap-pair rhs: [64, 2, 512], pair stride 2 elems
                v = S[64 * h:64 * (h + 1), base:base + 1].unsqueeze(1)
                v.ap = bass_rust.VecI64Pair([[SW, 64], [2, 2], [1, 512]])
                return v

            # rhs source (tile-sel, col offset) per (pair, parity)
            # parity 0 (even outputs), parity 1 (odd outputs)
            RSRC = [
                # P0        P1          P2          P3
                [("E", 0), ("E", 1), ("O", 0), ("O", 1)],
                [("O", 0), ("O", 1), ("E", 1), ("E", 2)],
            ]

            binq = []   # queued (emit_fn) for next pair's input prep

            def emit_binarize(p, c0, SEt, SOt):
                act_pair = p < ACT_BIN_PAIRS
                F = fpool.tile([128, SIGN_CHUNK], f32, tag="F")
                nc.sync.dma_start(
                    out=F[:, :],
                    in_=I_in[2 * p:2 * p + 2, :, c0:c0 + SIGN_CHUNK]
                    .rearrange("b ci l -> (b ci) l"))
                Fv = F[:].rearrange("pp (n two) -> pp n two", two=2)
                half = SIGN_CHUNK // 2
                se = SEt[:, c0 // 2 + 2:c0 // 2 + 2 + half]
                so = SOt[:, c0 // 2 + 1:c0 // 2 + 1 + half]
                if act_pair:
                    nc.scalar.activation(out=se, in_=Fv[:, :, 1], func=AF.Sign)
                    nc.scalar.activation(out=so, in_=Fv[:, :, 0], func=AF.Sign)
                else:
                    nc.vector.tensor_scalar(out=se, in0=Fv[:, :, 1],
                                            scalar1=0.0, scalar2=None,
                                            op0=OP.is_ge)
                    nc.vector.tensor_scalar(out=so, in0=Fv[:, :, 0],
                                            scalar1=0.0, scalar2=None,
                                            op0=OP.is_ge)

            def new_stiles(p):
                act_pair = p < ACT_BIN_PAIRS
                padv = -1.0 if act_pair else 0.0
                SEt = spool.tile([128, SW], fp8, tag="SE")
                SOt = spool.tile([128, SW], fp8, tag="SO")
                nc.vector.memset(SEt[:, 0:2], padv)
                nc.vector.memset(SEt[:, 8194:SW], padv)
                nc.vector.memset(SOt[:, 0:1], padv)
                nc.vector.memset(SOt[:, 8193:SW], padv)
                return SEt, SOt

            # prologue: pair 0 input prep
            S_cur = new_stiles(0)
            for c0 in range(0, L, SIGN_CHUNK):
                emit_binarize(0, c0, *S_cur)

            for p in range(PAIRS):
                SEt, SOt = S_cur
                ebias = thr[:, 0:1] if p < ACT_BIN_PAIRS else bias_d[:, 0:1]

                # CE/CO (+-1 bf16) and T per batch
                CEO = [cpool.tile([128, 2, CW], bf16, tag="CEO",
                                  name=f"CEO{h}_{p}")
                       for h in range(2)]
                Tb = [tpool.tile([128, NT], bf16, tag="T", name=f"T{h}_{p}")
                      for h in range(2)]
                for h in range(2):
                    nc.vector.memset(CEO[h][:, 0, 8192:8193], -1.0)

                if p + 1 < PAIRS:
                    S_nxt = new_stiles(p + 1)
                else:
                    S_nxt = None

                st_done = [0, 0]

                def emit_stage1(h, c):
                    X = xpool.tile([128, 2048], bf16, tag="X")
                    ce = CEO[h][:, 0, :]
                    co = CEO[h][:, 1, :]
                    nc.vector.tensor_tensor(
                        out=X[:, :], in0=ce[:, 2048 * c:2048 * c + 2048],
                        in1=co[:, 2048 * c:2048 * c + 2048], op=OP.max)
                    nc.vector.tensor_tensor(
                        out=Tb[h][:, 2048 * c:2048 * c + 2048], in0=X[:, :],
                        in1=ce[:, 2048 * c + 1:2048 * c + 2049], op=OP.max)

                for rr in range(NREG):
                    for h in range(2):
                        for P in range(4):
                            lw = w8[64 * h:64 * (h + 1),
                                    256 * P:256 * P + 256].rearrange(
                                        "p (two m) -> p two m", two=2)
                            for par in range(2):
                                src, off = RSRC[par][P]
                                S = SEt if src == "E" else SOt
                                for w in range(2):
                                    i0 = 1024 * rr + 512 * w
                                    nc.tensor.matmul(
                                        PS[:, 2048 * h + 1024 * w
                                           + 512 * par:
                                           2048 * h + 1024 * w
                                           + 512 * par + 512],
                                        lw, rhs_ap(S, h, i0 + off),
                                        start=(P == 0), stop=(P == 3),
                                        perf_mode=PM.DoubleRow)
                        # evac the OTHER slots were filled last iter;
                        # evac this half's region now (overlaps next
                        # half's matmuls in the PE queue)
                        pin = PS[:, 2048 * h:2048 * h + 2048].rearrange(
                            "p (g pr c) -> p pr g c", g=2, pr=2, c=512)
                        pout = CEO[h][:, :, 1024 * rr:1024 * rr
                                      + 1024].rearrange(
                            "p pr (g c) -> p pr g c", g=2, c=512)
                        nc.scalar.activation(out=pout, in_=pin, func=AF.Sign,
                                             bias=ebias)
                    # interleave next pair's input prep
                    if S_nxt is not None and rr % 2 == 1 and rr // 2 < 4:
                        emit_binarize(p + 1, (rr // 2) * SIGN_CHUNK, *S_nxt)
                    # stage-1 chunks lag the evacs they depend on
                    for h in range(2):
                        c = st_done[h]
                        if c < 3 and rr >= 2 * c + 2:
                            emit_stage1(h, c)
                            st_done[h] += 1

                # last stage-1 chunks + tails + store
                for h in range(2):
                    for c in range(st_done[h], 4):
                        emit_stage1(h, c)
                    V1 = vpool.tile([128, 8190], bf16, tag="V1")
                    nc.vector.tensor_tensor(out=V1[:, :],
                                            in0=Tb[h][:, 0:8190],
                                            in1=Tb[h][:, 1:8191], op=OP.max)
                    Ofin = opool.tile([128, Lp], bf16, tag="Ofin")
                    nc.vector.tensor_tensor(out=Ofin[:, :],
                                            in0=V1[:, 0:Lp],
                                            in1=Tb[h][:, 2:2 + Lp], op=OP.max)
                    nc.sync.dma_start(out=O_out[2 * p + h], in_=Ofin[:, :])

                S_cur = S_nxt

    nc.compile()
    return nc


def _build_legacy(fast: bool, unit: bool):
    """Baseline kernel (pool -> threshold), used for non-unit params."""
    import concourse.mybir as mybir
    from concourse import bacc
    from concourse.tile import TileContext

    f32 = mybir.dt.float32
    bf16 = mybir.dt.bfloat16
    AF = mybir.ActivationFunctionType
    OP = mybir.AluOpType

    LNT = Lp + 3
    NPAR = LPAD // 2
    GROUP = 1024
    GSTRIDE = GROUP - 2
    NGROUPS = 16
    TAIL_S = 16352
    TAIL_W = 32
    ACT_THRESH_BATCHES = 6
    EVAC_DVE_MOD = 5

    nc = bacc.Bacc()
    I_in = nc.declare_dram_parameter("I", [BPC, Cin, L], f32, isOutput=False)
    W_in = nc.declare_dram_parameter("W", [Cin, K * Cout], f32,
                                     isOutput=False)
    thr_in = nc.declare_dram_parameter("thr", [Cout, 8], f32, isOutput=False)
    O_out = nc.declare_dram_parameter("O", [BPC, Cout, Lp], bf16,
                                      isOutput=True)

    with TileContext(nc) as tc:
        with (
            tc.tile_pool(name="wpool", bufs=1) as wpool,
            tc.tile_pool(name="spool", bufs=2 if fast else 1) as spool,
            tc.tile_pool(name="fpool", bufs=2) as fpool,
            tc.tile_pool(name="tpool", bufs=2) as tpool,
            tc.tile_pool(name="vpool", bufs=2 if fast else 1) as vpool,
            tc.tile_pool(name="opool", bufs=2) as opool,
            tc.tile_pool(name="gpool", bufs=1) as gpool,
            tc.tile_pool(name="cepool", bufs=4) as cepool,
            tc.tile_pool(name="rpool", bufs=2) as rpool,
            tc.tile_pool(name="pspool", bufs=8, space="PSUM") as pspool,
        ):
            wf = wpool.tile([128, K * Cout], f32, tag="wf")
            nc.sync.dma_start(out=wf[0:64, :], in_=W_in[:])
            nc.sync.dma_start(out=wf[64:128, :], in_=W_in[:])
            wb = wpool.tile([128, K * Cout], bf16, tag="wb")
            nc.scalar.activation(out=wb[:, :], in_=wf[:, :], func=AF.Sign)

            thr = wpool.tile([128, 8], f32, tag="thr")
            nc.sync.dma_start(out=thr[:, :], in_=thr_in[:])

            groups = [(g * GSTRIDE, GROUP, g * (GROUP // 2 - 1))
                      for g in range(NGROUPS)]
            groups.append((TAIL_S, TAIL_W, NGROUPS * (GROUP // 2 - 1)))

            batch_idx = 0
            for p in range(PAIRS):
                SEt = spool.tile([128, NPAR], bf16, tag="SE")
                SOt = spool.tile([128, NPAR], bf16, tag="SO")
                for c0 in range(0, L, SIGN_CHUNK):
                    F = fpool.tile([128, SIGN_CHUNK], f32, tag="F")
                    nc.sync.dma_start(
                        out=F[:, :],
                        in_=I_in[2 * p:2 * p + 2, :, c0:c0 + SIGN_CHUNK]
                        .rearrange("b ci l -> (b ci) l"))
                    Fv = F[:].rearrange("p (n two) -> p n two", two=2)
                    half = SIGN_CHUNK // 2
                    nc.scalar.activation(
                        out=SEt[:, c0 // 2 + 2:c0 // 2 + 2 + half],
                        in_=Fv[:, :, 1], func=AF.Sign)
                    nc.scalar.activation(
                        out=SOt[:, c0 // 2 + 1:c0 // 2 + 1 + half],
                        in_=Fv[:, :, 0], func=AF.Sign)
                nc.vector.memset(SEt[:, 0:2], -1.0)
                nc.vector.memset(SEt[:, NPAR - 1:NPAR], -1.0)
                nc.vector.memset(SOt[:, 0:1], -1.0)
                nc.vector.memset(SOt[:, NPAR - 2:NPAR], -1.0)

                Tlo = tpool.tile([128, LNT], bf16, tag="T")
                Thi = tpool.tile([128, LNT], bf16, tag="T")

                def rhs(par, half, s, tap, n):
                    if par == 0:
                        src, n0 = (SEt, (s + tap) // 2) if tap % 2 == 0 \
                            else (SOt, (s + tap - 1) // 2)
                    else:
                        src, n0 = (SOt, (s + tap) // 2) if tap % 2 == 0 \
                            else (SEt, (s + tap + 1) // 2)
                    return src[64 * half:64 * (half + 1), n0:n0 + n]

                for gi, (s, w, t0) in enumerate(groups):
                    h = w // 2
                    pse = [pspool.tile([128, h], f32, tag="ps",
                                       name=f"pse{i}_{p}_{s}")
                           for i in range(2)]
                    pso = [pspool.tile([128, h], f32, tag="ps",
                                       name=f"pso{i}_{p}_{s}")
                           for i in range(2)]
                    for tap in range(K):
                        st = (tap == 0)
                        sp = (tap == K - 1)
                        for half in range(2):
                            lw = wb[64 * half:64 * (half + 1),
                                    tap * Cout:(tap + 1) * Cout]
                            nc.tensor.matmul(
                                pse[half][:, 0:h], lw, rhs(0, half, s, tap, h),
                                start=st, stop=sp)
                            nc.tensor.matmul(
                                pso[half][:, 0:h], lw, rhs(1, half, s, tap, h),
                                start=st, stop=sp)
                    for (half, Tbx) in ((0, Tlo), (1, Thi)):
                        CE = cepool.tile([128, 520], bf16, tag="CE")
                        if (2 * gi + half) % EVAC_DVE_MOD == 0:
                            nc.vector.tensor_copy(out=CE[:, 0:h],
                                                  in_=pse[half][:, 0:h])
                        else:
                            nc.scalar.activation(out=CE[:, 0:h],
                                                 in_=pse[half][:, 0:h],
                                                 func=AF.Copy)
                        nc.vector.memset(CE[:, h:h + 2], 0.0)
                        R = rpool.tile([128, 512], bf16, tag="R")
                        nc.vector.tensor_tensor(
                            out=R[:, 0:h], in0=CE[:, 0:h],
                            in1=pso[half][:, 0:h], op=OP.max)
                        nc.vector.tensor_tensor(
                            out=Tbx[:, t0:t0 + h], in0=R[:, 0:h],
                            in1=CE[:, 1:h + 1], op=OP.max)

                for (b, Tbx) in ((2 * p, Tlo), (2 * p + 1, Thi)):
                    on_act = batch_idx >= (BPC - ACT_THRESH_BATCHES)
                    batch_idx += 1
                    V = vpool.tile([128, Lp + 1], bf16, tag="V")
                    Ofin = opool.tile([128, Lp + 1], bf16, tag="Ofin")
                    nc.vector.tensor_tensor(out=V[:, 0:Lp + 1],
                                            in0=Tbx[:, 0:Lp + 1],
                                            in1=Tbx[:, 1:Lp + 2], op=OP.max)
                    nc.vector.tensor_tensor(out=V[:, 0:Lp + 1],
                                            in0=V[:, 0:Lp + 1],
                                            in1=Tbx[:, 2:Lp + 3], op=OP.max)
                    if fast:
                        if on_act:
                            nc.scalar.activation(out=Ofin[:, :], in_=V[:, :],
                                                 func=AF.Sign,
                                                 bias=thr[:, 0:1])
                            if not unit:
                                nc.vector.tensor_scalar(
                                    out=Ofin[:, :], in0=Ofin[:, :],
                                    scalar1=thr[:, 4:5], scalar2=None,
                                    op0=OP.mult)
                        else:
                            s2 = 2.0 if unit else thr[:, 3:4]
                            s3 = 1.0 if unit else thr[:, 4:5]
                            nc.vector.tensor_scalar(
                                out=V[:, :], in0=V[:, :], scalar1=thr[:, 1:2],
                                scalar2=s2, op0=OP.is_gt, op1=OP.mult)
                            nc.vector.tensor_scalar(
                                out=Ofin[:, :], in0=V[:, :], scalar1=s3,
                                scalar2=None, op0=OP.subtract)
                    else:
                        G = gpool.tile([128, Lp + 1], bf16, tag="G")
                        Gn = gpool.tile([128, Lp + 1], bf16, tag="Gn")
                        G0 = gpool.tile([128, Lp + 1], bf16, tag="G0")
                        nc.vector.tensor_scalar(
                            out=G[:, :], in0=V[:, :], scalar1=thr[:, 1:2],
                            scalar2=thr[:, 3:4], op0=OP.is_gt, op1=OP.mult)
                        nc.vector.tensor_scalar(
                            out=G[:, :], in0=G[:, :], scalar1=thr[:, 4:5],
                            scalar2=None, op0=OP.subtract)
                        nc.vector.tensor_scalar(
                            out=Gn[:, :], in0=V[:, :], scalar1=thr[:, 2:3],
                            scalar2=thr[:, 5:6], op0=OP.is_gt, op1=OP.mult)
                        nc.vector.tensor_scalar(
                            out=Gn[:, :], in0=Gn[:, :], scalar1=thr[:, 6:7],
                            scalar2=None, op0=OP.subtract)
                        nc.vector.tensor_scalar(
                            out=G0[:, :], in0=V[:, :], scalar1=0.0,
                            scalar2=None, op0=OP.is_ge)
                        nc.vector.tensor_tensor(out=G[:, :], in0=G[:, :],
                                                in1=Gn[:, :], op=OP.subtract)
                        nc.vector.tensor_tensor(out=G[:, :], in0=G0[:, :],
                                                in1=G[:, :], op=OP.mult)
                        nc.vector.tensor_tensor(out=Ofin[:, :], in0=G[:, :],
                                                in1=Gn[:, :], op=OP.add)
                    nc.sync.dma_start(out=O_out[b], in_=Ofin[:, 0:Lp])

    nc.compile()
    return nc


def _get_nc(fast, unit):
    key = (fast, unit)
    if key not in _CACHE:
        if unit:
            _CACHE[key] = _build_v2()
        else:
            _CACHE[key] = _build_legacy(fast, unit)
    return _CACHE[key]


def prep_weights_v2(W):
    """[Cout, Cin, K] f32 -> [Cin, 7*128] slot layout t0,t4,t2,t6,t1,t5,t3."""
    slot_taps = [0, 4, 2, 6, 1, 5, 3]
    Wt = W.transpose(1, 2, 0)  # [Cin, K, Cout]
    return np.ascontiguousarray(
        Wt[:, slot_taps, :].reshape(Cin, 7 * Cout))


def prep_inputs(inputs):
    """Host-side layout prep shared by kernel() and test harnesses.

    Returns (unit, fast, in_maps) where in_maps feeds
    run_bass_kernel_spmd for the build selected by (fast, unit).
    """
    tp = np.asarray(inputs["threshold_plus"], dtype=np.float32)
    tm = np.asarray(inputs["threshold_minus"], dtype=np.float32)
    ps = np.asarray(inputs["threshold_plus_sign"], dtype=np.float32)
    ms = np.asarray(inputs["threshold_minus_sign"], dtype=np.float32)
    I = np.ascontiguousarray(np.asarray(inputs["I"], dtype=np.float32))
    W = np.asarray(inputs["W"], dtype=np.float32)

    fast = np.array_equal(tp, tm) and np.array_equal(ps, ms)
    unit = fast and bool(np.all(ps == 1.0)) and bool(np.all(tp == 1.0))

    thr = np.zeros((Cout, 8), dtype=np.float32)
    if unit:
        thr[:, 0] = -tp
        thr[:, 1] = -0.5 * tp
        Wd = prep_weights_v2(W)
    else:
        thr[:, 0] = -tp
        thr[:, 1] = tp
        thr[:, 2] = tm
        thr[:, 3] = 2.0 * ps
        thr[:, 4] = ps
        thr[:, 5] = 2.0 * ms
        thr[:, 6] = ms
        Wd = np.ascontiguousarray(
            W.transpose(1, 2, 0).reshape(Cin, K * Cout))

    in_maps = [
        {"I": I[c * BPC:(c + 1) * BPC], "W": Wd, "thr": thr}
        for c in range(NCORES)
    ]
    return unit, fast, in_maps


def kernel(I, W, threshold_plus, threshold_minus, threshold_plus_sign,
           threshold_minus_sign):
    from concourse.bass_utils import run_bass_kernel_spmd

    unit, fast, in_maps = prep_inputs({
        "I": I, "W": W,
        "threshold_plus": threshold_plus,
        "threshold_minus": threshold_minus,
        "threshold_plus_sign": threshold_plus_sign,
        "threshold_minus_sign": threshold_minus_sign,
    })
    nc = _get_nc(fast, unit)
    res = run_bass_kernel_spmd(nc, in_maps, list(range(NCORES)))
    out = np.concatenate(
        [np.asarray(r["O"]).astype(np.float32) for r in res.results], axis=0)
    return out


# revision 10
# speedup vs baseline: 1.0086x; 1.0086x over previous
"""Binary conv1d + maxpool + per-channel threshold, Trainium2 Bass kernel.

Problem (hardcoded shapes):
  I:  [64, 64, 16384] f32   -> pad L by (3,3) with -1.0, sign()
  W:  [128, 64, 7]    f32   -> sign()
  conv1d (VALID over padded) -> [64, 128, 16384]
  maxpool1d(k=7, s=2)        -> [64, 128, 8189]
  per-channel threshold      -> +-sign outputs

Sharding: data-parallel over batch, 8 batches per core on 8 cores.

v2 device algorithm (unit-threshold fast path), per core (4 batch pairs):
  - Binarize to fp8 parity tiles SE/SO (batch pair stacked on 128
    partitions).  Two pairs binarize on ScalarE (Sign, +-1); two pairs on
    DVE (is_ge, {0,1}) to balance engine load.  The {0,1} encoding is
    corrected via a per-channel threshold shift (conv = 2*conv01 - W1).
  - Conv: fp8 DoubleRow matmuls, 2 taps per MM via pair-stride-2 rhs APs
    (taps {0,4},{2,6},{1,5},{3,zero}); 4 weight pairs x 2 parities x 2
    512-col windows per 2048-col super-region, 2 batches concurrent on
    the PE row-tile halves.  PSUM is one manual [128,4096] f32 tile,
    halves alternate so evacuation overlaps the other half's matmuls.
  - Evac fuses the threshold: ScalarE Sign(psum + bias) -> +-1 bf16 into
    CE/CO tiles (threshold commutes with max since f is monotone for the
    unit params).  2048-wide evac ops with a parity-splitting 4D out AP.
  - Pool: T[i] = max(ce[i], co[i], ce[i+1]) as 2 DVE 2048-col TT maxes;
    tail out[l] = max(T[l],T[l+1],T[l+2]) as 2 full-width TT maxes.
    All +-1 bf16, so the pooled value IS the final output (no threshold
    pass, no final map).
  - Output bf16, cast to f32 on host.
"""

import numpy as np

B, Cin, L = 64, 64, 16384
Cout, K = 128, 7
PAD = 3
LPAD = L + 2 * PAD          # 16390
Lp = (L - 7) // 2 + 1       # 8189
NCORES = 8
BPC = B // NCORES           # 8 batches per core
PAIRS = BPC // 2            # 4

SW = 8196                   # SE/SO fp8 tile width (8195 data + 1 pad)
CW = 8200                   # CE/CO region pitch in the CEO tile
NT = 8192                   # T-buffer slots per batch
NREG = 8                    # 2048-conv-col super-regions per batch
SIGN_CHUNK = 4096
ACT_BIN_PAIRS = 2           # pairs 0..1 binarize on ScalarE (+-1)

_CACHE = {}


def _build_v2():
    import concourse.mybir as mybir
    import bass_rust
    from concourse import bacc
    from concourse.bass import AP
    from concourse.tile import TileContext

    f32 = mybir.dt.float32
    bf16 = mybir.dt.bfloat16
    fp8 = mybir.dt.float8e4
    AF = mybir.ActivationFunctionType
    OP = mybir.AluOpType
    PM = mybir.MatmulPerfMode

    nc = bacc.Bacc()
    I_in = nc.declare_dram_parameter("I", [BPC, Cin, L], f32, isOutput=False)
    # W host layout: [Cin, 7 slots x 128] f32, slot order t0,t4,t2,t6,t1,t5,t3
    W_in = nc.declare_dram_parameter("W", [Cin, 896], f32, isOutput=False)
    # thr: col0 = -tp, col1 = -0.5*tp
    thr_in = nc.declare_dram_parameter("thr", [Cout, 8], f32, isOutput=False)
    O_out = nc.declare_dram_parameter("O", [BPC, Cout, Lp], bf16, isOutput=True)

    with TileContext(nc) as tc:
        with (
            tc.tile_pool(name="wpool", bufs=1) as wpool,
            tc.tile_pool(name="fpool", bufs=2) as fpool,
            tc.tile_pool(name="spool", bufs=2) as spool,
            tc.tile_pool(name="cpool", bufs=1) as cpool,
            tc.tile_pool(name="tpool", bufs=1) as tpool,
            tc.tile_pool(name="opool", bufs=2) as opool,
            tc.tile_pool(name="pspool", bufs=1, space="PSUM") as pspool,
        ):
            # ---- weights: sign -> fp8, 7 real slots + 1 zero slot
            wf = fpool.tile([128, SIGN_CHUNK], f32, tag="F", name="wf")
            nc.sync.dma_start(out=wf[0:64, 0:896], in_=W_in[:])
            nc.sync.dma_start(out=wf[64:128, 0:896], in_=W_in[:])
            w8 = wpool.tile([128, 1024], fp8, tag="w8")
            nc.scalar.activation(out=w8[:, 0:896], in_=wf[:, 0:896],
                                 func=AF.Sign)
            nc.vector.memset(w8[:, 896:1024], 0.0)

            thr = wpool.tile([128, 8], f32, tag="thr")
            nc.sync.dma_start(out=thr[:, :], in_=thr_in[:])

            ones8 = wpool.tile([128, 1], fp8, tag="ones8")
            nc.vector.memset(ones8[:, :], 1.0)

            PS = pspool.tile([128, 4096], f32, tag="PS", name="PS")

            # ---- W1[co] = sum_{ci,k} sign(w): 7 accumulating 1-col matmuls
            for k in range(7):
                nc.tensor.matmul(PS[:, 0:1],
                                 w8[0:64, 128 * k:128 * (k + 1)],
                                 ones8[0:64, 0:1],
                                 start=(k == 0), stop=(k == 6))
            # bias_d = -0.5*W1 - 0.5*tp  (evac bias for {0,1}-encoded pairs)
            bias_d = wpool.tile([128, 1], f32, tag="bias_d")
            nc.vector.tensor_scalar(out=bias_d[:, :], in0=PS[:, 0:1],
                                    scalar1=-0.5, scalar2=thr[:, 1:2],
                                    op0=OP.mult, op1=OP.add)

            # weight slot layout: pair P cols [256P : 256P+256] as [2 x 128]
            # P0=(t0,t4) slots 0,1; P1=(t2,t6) slots 2,3; P2=(t1,t5) slots
            # 4,5; P3=(t3,zero) slots 6,7.

            def rhs_ap(S, h, base):
                # sliding tap-pair rhs: [64, 2, 512], pair stride 2 elems
                v = S[64 * h:64 * (h + 1), base:base + 1].unsqueeze(1)
                v.ap = bass_rust.VecI64Pair([[SW, 64], [2, 2], [1, 512]])
                return v

            # rhs source (tile-sel, col offset) per (parity, pair)
            RSRC = [
                [("E", 0), ("E", 1), ("O", 0), ("O", 1)],
                [("O", 0), ("O", 1), ("E", 1), ("E", 2)],
            ]

            def emit_binarize(p, c0, SEt, SOt):
                act_pair = p < ACT_BIN_PAIRS
                F = fpool.tile([128, SIGN_CHUNK], f32, tag="F",
                               name=f"F{p}_{c0}")
                nc.sync.dma_start(
                    out=F[:, :],
                    in_=I_in[2 * p:2 * p + 2, :, c0:c0 + SIGN_CHUNK]
                    .rearrange("b ci l -> (b ci) l"))
                Fv = F[:].rearrange("pp (n two) -> pp n two", two=2)
                half = SIGN_CHUNK // 2
                se = SEt[:, c0 // 2 + 2:c0 // 2 + 2 + half]
                so = SOt[:, c0 // 2 + 1:c0 // 2 + 1 + half]
                if act_pair:
                    nc.scalar.activation(out=se, in_=Fv[:, :, 1], func=AF.Sign)
                    nc.scalar.activation(out=so, in_=Fv[:, :, 0], func=AF.Sign)
                else:
                    nc.vector.tensor_scalar(out=se, in0=Fv[:, :, 1],
                                            scalar1=0.0, scalar2=None,
                                            op0=OP.is_ge)
                    nc.vector.tensor_scalar(out=so, in0=Fv[:, :, 0],
                                            scalar1=0.0, scalar2=None,
                                            op0=OP.is_ge)

            def new_stiles(p):
                act_pair = p < ACT_BIN_PAIRS
                padv = -1.0 if act_pair else 0.0
                SEt = spool.tile([128, SW], fp8, tag="SE", name=f"SE{p}")
                SOt = spool.tile([128, SW], fp8, tag="SO", name=f"SO{p}")
                nc.vector.memset(SEt[:, 0:2], padv)
                nc.vector.memset(SEt[:, 8194:SW], padv)
                nc.vector.memset(SOt[:, 0:1], padv)
                nc.vector.memset(SOt[:, 8193:SW], padv)
                return SEt, SOt

            # prologue: pair 0 input prep
            S_cur = new_stiles(0)
            for c0 in range(0, L, SIGN_CHUNK):
                emit_binarize(0, c0, *S_cur)

            for p in range(PAIRS):
                SEt, SOt = S_cur
                ebias = thr[:, 0:1] if p < ACT_BIN_PAIRS else bias_d[:, 0:1]

                # CE/CO (+-1 bf16): [h, parity, col]; T: [h, col]
                CEO = cpool.tile([128, 2, 2, CW], bf16, tag="CEO",
                                 name=f"CEO{p}")
                Tb = tpool.tile([128, 2, NT], bf16, tag="T", name=f"T{p}")
                nc.vector.memset(CEO[:, :, 0, 8192:8193], -1.0)

                if p + 1 < PAIRS:
                    S_nxt = new_stiles(p + 1)
                else:
                    S_nxt = None

                st_done = [0]

                def emit_stage1(c):
                    # T[h, i] = max(ce[h,i], co[h,i], ce[h,i+1]), both h in
                    # one op; second max runs in place on T
                    cs = slice(2048 * c, 2048 * c + 2048)
                    nc.vector.tensor_tensor(
                        out=Tb[:, :, cs], in0=CEO[:, :, 0, cs],
                        in1=CEO[:, :, 1, cs], op=OP.max)
                    nc.vector.tensor_tensor(
                        out=Tb[:, :, cs], in0=Tb[:, :, cs],
                        in1=CEO[:, :, 0, 2048 * c + 1:2048 * c + 2049],
                        op=OP.max)

                for rr in range(NREG):
                    for w in range(2):
                        # 16 MMs per window, halves innermost-alternating
                        for P in range(4):
                            for par in range(2):
                                src, off = RSRC[par][P]
                                S = SEt if src == "E" else SOt
                                i0 = 1024 * rr + 512 * w
                                for h in range(2):
                                    lw = w8[64 * h:64 * (h + 1),
                                            256 * P:256 * P + 256].rearrange(
                                                "p (two m) -> p two m", two=2)
                                    nc.tensor.matmul(
                                        PS[:, 2048 * h + 1024 * w
                                           + 512 * par:
                                           2048 * h + 1024 * w
                                           + 512 * par + 512],
                                        lw, rhs_ap(S, h, i0 + off),
                                        start=(P == 0), stop=(P == 3),
                                        perf_mode=PM.DoubleRow)
                        # one evac op for both halves of this window;
                        # runs while the other window's MMs stream
                        pin = AP(tensor=PS[:, :].tensor, offset=1024 * w,
                                 ap=bass_rust.VecI64Pair(
                                     [[4096, 128], [2048, 2], [512, 2],
                                      [1, 512]]))
                        c0 = 1024 * rr + 512 * w
                        nc.scalar.activation(out=CEO[:, :, :, c0:c0 + 512],
                                             in_=pin, func=AF.Sign,
                                             bias=ebias)
                    # interleave next pair's input prep
                    if S_nxt is not None and rr % 2 == 1 and rr // 2 < 4:
                        emit_binarize(p + 1, (rr // 2) * SIGN_CHUNK, *S_nxt)
                    # stage-1 chunks lag the evacs they depend on
                    c = st_done[0]
                    if c < 3 and rr >= 2 * c + 2:
                        emit_stage1(c)
                        st_done[0] += 1

                # last stage-1 chunk + tails + store
                for c in range(st_done[0], 4):
                    emit_stage1(c)
                for h in range(2):
                    # V1 in place: T[l] = max(T[l], T[l+1]); then
                    # out[l] = max(V1[l], V1[l+1]) = max pool k=3
                    nc.vector.tensor_tensor(out=Tb[:, h, 0:8190],
                                            in0=Tb[:, h, 0:8190],
                                            in1=Tb[:, h, 1:8191], op=OP.max)
                    Ofin = opool.tile([128, Lp], bf16, tag="Ofin",
                                      name=f"Of{p}_{h}")
                    nc.vector.tensor_tensor(out=Ofin[:, :],
                                            in0=Tb[:, h, 0:Lp],
                                            in1=Tb[:, h, 1:1 + Lp], op=OP.max)
                    nc.sync.dma_start(out=O_out[2 * p + h], in_=Ofin[:, :])

                S_cur = S_nxt

    nc.compile()
    return nc


def _build_legacy(fast: bool, unit: bool):
    """Baseline kernel (pool -> threshold), used for non-unit params."""
    import concourse.mybir as mybir
    from concourse import bacc
    from concourse.tile import TileContext

    f32 = mybir.dt.float32
    bf16 = mybir.dt.bfloat16
    AF = mybir.ActivationFunctionType
    OP = mybir.AluOpType

    LNT = Lp + 3
    NPAR = LPAD // 2
    GROUP = 1024
    GSTRIDE = GROUP - 2
    NGROUPS = 16
    TAIL_S = 16352
    TAIL_W = 32
    ACT_THRESH_BATCHES = 6
    EVAC_DVE_MOD = 5

    nc = bacc.Bacc()
    I_in = nc.declare_dram_parameter("I", [BPC, Cin, L], f32, isOutput=False)
    W_in = nc.declare_dram_parameter("W", [Cin, K * Cout], f32,
                                     isOutput=False)
    thr_in = nc.declare_dram_parameter("thr", [Cout, 8], f32, isOutput=False)
    O_out = nc.declare_dram_parameter("O", [BPC, Cout, Lp], bf16,
                                      isOutput=True)

    with TileContext(nc) as tc:
        with (
            tc.tile_pool(name="wpool", bufs=1) as wpool,
            tc.tile_pool(name="spool", bufs=2 if fast else 1) as spool,
            tc.tile_pool(name="fpool", bufs=2) as fpool,
            tc.tile_pool(name="tpool", bufs=2) as tpool,
            tc.tile_pool(name="vpool", bufs=2 if fast else 1) as vpool,
            tc.tile_pool(name="opool", bufs=2) as opool,
            tc.tile_pool(name="gpool", bufs=1) as gpool,
            tc.tile_pool(name="cepool", bufs=4) as cepool,
            tc.tile_pool(name="rpool", bufs=2) as rpool,
            tc.tile_pool(name="pspool", bufs=8, space="PSUM") as pspool,
        ):
            wf = wpool.tile([128, K * Cout], f32, tag="wf")
            nc.sync.dma_start(out=wf[0:64, :], in_=W_in[:])
            nc.sync.dma_start(out=wf[64:128, :], in_=W_in[:])
            wb = wpool.tile([128, K * Cout], bf16, tag="wb")
            nc.scalar.activation(out=wb[:, :], in_=wf[:, :], func=AF.Sign)

            thr = wpool.tile([128, 8], f32, tag="thr")
            nc.sync.dma_start(out=thr[:, :], in_=thr_in[:])

            groups = [(g * GSTRIDE, GROUP, g * (GROUP // 2 - 1))
                      for g in range(NGROUPS)]
            groups.append((TAIL_S, TAIL_W, NGROUPS * (GROUP // 2 - 1)))

            batch_idx = 0
            for p in range(PAIRS):
                SEt = spool.tile([128, NPAR], bf16, tag="SE")
                SOt = spool.tile([128, NPAR], bf16, tag="SO")
                for c0 in range(0, L, SIGN_CHUNK):
                    F = fpool.tile([128, SIGN_CHUNK], f32, tag="F")
                    nc.sync.dma_start(
                        out=F[:, :],
                        in_=I_in[2 * p:2 * p + 2, :, c0:c0 + SIGN_CHUNK]
                        .rearrange("b ci l -> (b ci) l"))
                    Fv = F[:].rearrange("p (n two) -> p n two", two=2)
                    half = SIGN_CHUNK // 2
                    nc.scalar.activation(
                        out=SEt[:, c0 // 2 + 2:c0 // 2 + 2 + half],
                        in_=Fv[:, :, 1], func=AF.Sign)
                    nc.scalar.activation(
                        out=SOt[:, c0 // 2 + 1:c0 // 2 + 1 + half],
                        in_=Fv[:, :, 0], func=AF.Sign)
                nc.vector.memset(SEt[:, 0:2], -1.0)
                nc.vector.memset(SEt[:, NPAR - 1:NPAR], -1.0)
                nc.vector.memset(SOt[:, 0:1], -1.0)
                nc.vector.memset(SOt[:, NPAR - 2:NPAR], -1.0)

                Tlo = tpool.tile([128, LNT], bf16, tag="T")
                Thi = tpool.tile([128, LNT], bf16, tag="T")

                def rhs(par, half, s, tap, n):
                    if par == 0:
                        src, n0 = (SEt, (s + tap) // 2) if tap % 2 == 0 \
                            else (SOt, (s + tap - 1) // 2)
                    else:
                        src, n0 = (SOt, (s + tap) // 2) if tap % 2 == 0 \
                            else (SEt, (s + tap + 1) // 2)
                    return src[64 * half:64 * (half + 1), n0:n0 + n]

                for gi, (s, w, t0) in enumerate(groups):
                    h = w // 2
                    pse = [pspool.tile([128, h], f32, tag="ps",
                                       name=f"pse{i}_{p}_{s}")
                           for i in range(2)]
                    pso = [pspool.tile([128, h], f32, tag="ps",
                                       name=f"pso{i}_{p}_{s}")
                           for i in range(2)]
                    for tap in range(K):
                        st = (tap == 0)
                        sp = (tap == K - 1)
                        for half in range(2):
                            lw = wb[64 * half:64 * (half + 1),
                                    tap * Cout:(tap + 1) * Cout]
                            nc.tensor.matmul(
                                pse[half][:, 0:h], lw, rhs(0, half, s, tap, h),
                                start=st, stop=sp)
                            nc.tensor.matmul(
                                pso[half][:, 0:h], lw, rhs(1, half, s, tap, h),
                                start=st, stop=sp)
                    for (half, Tbx) in ((0, Tlo), (1, Thi)):
                        CE = cepool.tile([128, 520], bf16, tag="CE")
                        if (2 * gi + half) % EVAC_DVE_MOD == 0:
                            nc.vector.tensor_copy(out=CE[:, 0:h],
                                                  in_=pse[half][:, 0:h])
                        else:
                            nc.scalar.activation(out=CE[:, 0:h],
                                                 in_=pse[half][:, 0:h],
                                                 func=AF.Copy)
                        nc.vector.memset(CE[:, h:h + 2], 0.0)
                        R = rpool.tile([128, 512], bf16, tag="R")
                        nc.vector.tensor_tensor(
                            out=R[:, 0:h], in0=CE[:, 0:h],
                            in1=pso[half][:, 0:h], op=OP.max)
                        nc.vector.tensor_tensor(
                            out=Tbx[:, t0:t0 + h], in0=R[:, 0:h],
                            in1=CE[:, 1:h + 1], op=OP.max)

                for (b, Tbx) in ((2 * p, Tlo), (2 * p + 1, Thi)):
                    on_act = batch_idx >= (BPC - ACT_THRESH_BATCHES)
                    batch_idx += 1
                    V = vpool.tile([128, Lp + 1], bf16, tag="V")
                    Ofin = opool.tile([128, Lp + 1], bf16, tag="Ofin")
                    nc.vector.tensor_tensor(out=V[:, 0:Lp + 1],
                                            in0=Tbx[:, 0:Lp + 1],
                                            in1=Tbx[:, 1:Lp + 2], op=OP.max)
                    nc.vector.tensor_tensor(out=V[:, 0:Lp + 1],
                                            in0=V[:, 0:Lp + 1],
                                            in1=Tbx[:, 2:Lp + 3], op=OP.max)
                    if fast:
                        if on_act:
                            nc.scalar.activation(out=Ofin[:, :], in_=V[:, :],
                                                 func=AF.Sign,
                                                 bias=thr[:, 0:1])
                            if not unit:
                                nc.vector.tensor_scalar(
                                    out=Ofin[:, :], in0=Ofin[:, :],
                                    scalar1=thr[:, 4:5], scalar2=None,
                                    op0=OP.mult)
                        else:
                            s2 = 2.0 if unit else thr[:, 3:4]
                            s3 = 1.0 if unit else thr[:, 4:5]
                            nc.vector.tensor_scalar(
                                out=V[:, :], in0=V[:, :], scalar1=thr[:, 1:2],
                                scalar2=s2, op0=OP.is_gt, op1=OP.mult)
                            nc.vector.tensor_scalar(
                                out=Ofin[:, :], in0=V[:, :], scalar1=s3,
                                scalar2=None, op0=OP.subtract)
                    else:
                        G = gpool.tile([128, Lp + 1], bf16, tag="G")
                        Gn = gpool.tile([128, Lp + 1], bf16, tag="Gn")
                        G0 = gpool.tile([128, Lp + 1], bf16, tag="G0")
                        nc.vector.tensor_scalar(
                            out=G[:, :], in0=V[:, :], scalar1=thr[:, 1:2],
                            scalar2=thr[:, 3:4], op0=OP.is_gt, op1=OP.mult)
                        nc.vector.tensor_scalar(
                            out=G[:, :], in0=G[:, :], scalar1=thr[:, 4:5],
                            scalar2=None, op0=OP.subtract)
                        nc.vector.tensor_scalar(
                            out=Gn[:, :], in0=V[:, :], scalar1=thr[:, 2:3],
                            scalar2=thr[:, 5:6], op0=OP.is_gt, op1=OP.mult)
                        nc.vector.tensor_scalar(
                            out=Gn[:, :], in0=Gn[:, :], scalar1=thr[:, 6:7],
                            scalar2=None, op0=OP.subtract)
                        nc.vector.tensor_scalar(
                            out=G0[:, :], in0=V[:, :], scalar1=0.0,
                            scalar2=None, op0=OP.is_ge)
                        nc.vector.tensor_tensor(out=G[:, :], in0=G[:, :],
                                                in1=Gn[:, :], op=OP.subtract)
                        nc.vector.tensor_tensor(out=G[:, :], in0=G0[:, :],
                                                in1=G[:, :], op=OP.mult)
                        nc.vector.tensor_tensor(out=Ofin[:, :], in0=G[:, :],
                                                in1=Gn[:, :], op=OP.add)
                    nc.sync.dma_start(out=O_out[b], in_=Ofin[:, 0:Lp])

    nc.compile()
    return nc


def _get_nc(fast, unit):
    key = (fast, unit)
    if key not in _CACHE:
        if unit:
            _CACHE[key] = _build_v2()
        else:
            _CACHE[key] = _build_legacy(fast, unit)
    return _CACHE[key]


def prep_weights_v2(W):
    """[Cout, Cin, K] f32 -> [Cin, 7*128] slot layout t0,t4,t2,t6,t1,t5,t3."""
    slot_taps = [0, 4, 2, 6, 1, 5, 3]
    Wt = W.transpose(1, 2, 0)  # [Cin, K, Cout]
    return np.ascontiguousarray(
        Wt[:, slot_taps, :].reshape(Cin, 7 * Cout))


def prep_inputs(inputs):
    """Host-side layout prep shared by kernel() and test harnesses.

    Returns (unit, fast, in_maps) where in_maps feeds
    run_bass_kernel_spmd for the build selected by (fast, unit).
    """
    tp = np.asarray(inputs["threshold_plus"], dtype=np.float32)
    tm = np.asarray(inputs["threshold_minus"], dtype=np.float32)
    ps = np.asarray(inputs["threshold_plus_sign"], dtype=np.float32)
    ms = np.asarray(inputs["threshold_minus_sign"], dtype=np.float32)
    I = np.ascontiguousarray(np.asarray(inputs["I"], dtype=np.float32))
    W = np.asarray(inputs["W"], dtype=np.float32)

    fast = np.array_equal(tp, tm) and np.array_equal(ps, ms)
    unit = fast and bool(np.all(ps == 1.0)) and bool(np.all(tp == 1.0))

    thr = np.zeros((Cout, 8), dtype=np.float32)
    if unit:
        thr[:, 0] = -tp
        thr[:, 1] = -0.5 * tp
        Wd = prep_weights_v2(W)
    else:
        thr[:, 0] = -tp
        thr[:, 1] = tp
        thr[:, 2] = tm
        thr[:, 3] = 2.0 * ps
        thr[:, 4] = ps
        thr[:, 5] = 2.0 * ms
        thr[:, 6] = ms
        Wd = np.ascontiguousarray(
            W.transpose(1, 2, 0).reshape(Cin, K * Cout))

    in_maps = [
        {"I": I[c * BPC:(c + 1) * BPC], "W": Wd, "thr": thr}
        for c in range(NCORES)
    ]
    return unit, fast, in_maps


def kernel(I, W, threshold_plus, threshold_minus, threshold_plus_sign,
           threshold_minus_sign):
    from concourse.bass_utils import run_bass_kernel_spmd

    unit, fast, in_maps = prep_inputs({
        "I": I, "W": W,
        "threshold_plus": threshold_plus,
        "threshold_minus": threshold_minus,
        "threshold_plus_sign": threshold_plus_sign,
        "threshold_minus_sign": threshold_minus_sign,
    })
    nc = _get_nc(fast, unit)
    res = run_bass_kernel_spmd(nc, in_maps, list(range(NCORES)))
    out = np.concatenate(
        [np.asarray(r["O"]).astype(np.float32) for r in res.results], axis=0)
    return out
